# revision 35
# baseline (speedup 1.0000x reference)
"""Bidirectional attention kernel for Trainium2 (8 NeuronCores, batch-parallel).

Math (per batch element, all on one core):
    k1p = k1 @ W1 + b1            [N, A]
    k2p = k2 @ W2 + b2            [N, A]
    S   = k1p @ k2p.T             [N, N]
    E   = exp(S)                  (no max-subtraction needed: |S| < ~25)
    o1[m, d] = sum_n E[n, m] v1[n, d] / sum_n E[n, m]   (softmax over N1)
    o2[n, d] = sum_m E[n, m] v2[m, d] / sum_m E[n, m]   (softmax over N2)

Schedule (v2 — xbar-transpose design):
  * ET (the transpose of E, needed for o2) is produced by 16 DMA xbar
    slab transposes (dma_start_transpose on the ACT HWDGE ring), one per
    score row-tile, instead of 256 PE transposes + 80 DVE copies.  This
    removes ~14us of PE work and ~32us of DVE work.
  * DMA load order k1[rows 0:1024], k2, v2, k1[rows 1024:2048], v1 lets
    score row 0 start as soon as k1-half1 + k2 are projected, and v2e be
    ready for the first o2 groups.
  * exp on ACT paces the score phase (~1147ns per 1024-wide tile); the
    PE's spare time there is filled by o2 output groups, which chase the
    slab transposes with a 4-row slack.  o1 groups run after.
  * Projection bias-adds and output normalizes run on DVE (ACT does exp
    only); kT PSUM->SBUF copies run on ACT before the exp phase starts.
  * Normalizers are folded into the output matmuls by appending a ones
    column to v1/v2; the PSUM result's last column is reciprocated (DVE)
    and multiplied in (DVE), then stored.
"""

import numpy as np

import concourse.bass as bass
import concourse.tile as tile
from concourse import bacc, mybir, bass_utils
from concourse.masks import make_identity

N_CORES = 8
B = 8
N = 2048  # N1 == N2
KD = 256  # K1D == K2D
VD = 256  # V1D == V2D
AD = 128
P = 128

F32 = mybir.dt.float32
F32R = mybir.dt.float32r
BF16 = mybir.dt.bfloat16
AF = mybir.ActivationFunctionType

O2_SLACK = 6  # score rows between a slab transpose and the o2 group using it


def _emit_kprep(nc, stage, ktbuf, ptpp, k_d, W_sb, b_sb, kpT, identity, chunks):
    """Load, PE-transpose, and project 512-row chunks of one k tensor."""
    for c in chunks:
        st = stage.tile([P, 4, KD], F32, tag="stage", name="st")
        nc.sync.dma_start(
            out=st,
            in_=k_d[512 * c : 512 * (c + 1), :].rearrange("(t p) k -> p t k", p=P),
        )
        kt = ktbuf.tile([P, 2, 512], F32R, tag="kt", name="kt")
        for kb in range(2):
            pt = ptpp.tile([P, 512], F32, tag="pt512", name="pt")
            for t in range(4):
                nc.tensor.transpose(
                    pt[:, 128 * t : 128 * (t + 1)],
                    st[:, t, 128 * kb : 128 * (kb + 1)],
                    identity,
                )
            nc.vector.tensor_copy(kt[:, kb, :], pt)
        pp = ptpp.tile([P, 512], F32, tag="pt512", name="pp")
        for kb in range(2):
            nc.tensor.matmul(
                pp, lhsT=W_sb[:, kb, :], rhs=kt[:, kb, :],
                start=(kb == 0), stop=(kb == 1),
            )
        nc.vector.tensor_scalar_add(kpT[:, 512 * c : 512 * (c + 1)], pp, b_sb)


def _emit_vload(nc, stage, v_d, ve, nch, eng="gpsimd"):
    """Load one v tensor into its bf16 extended tile (ones column at VD)."""
    nc.gpsimd.memset(ve[:, :, VD : VD + 2], 1.0)
    copy_eng = getattr(nc, eng)
    for c in range(nch):
        sv = stage.tile([P, 4, VD], F32, tag="stage", name="sv")
        nc.sync.dma_start(
            out=sv,
            in_=v_d[512 * c : 512 * (c + 1), :].rearrange("(t p) d -> p t d", p=P),
        )
        copy_eng.tensor_copy(ve[:, 4 * c : 4 * (c + 1), 0:VD], sv)


class _OutBatcher:
    """Accumulates normalized output tiles and stores them 4-at-a-time with a
    single DMA (HWDGE dispatch overhead is ~625ns; 8 big stores beat 32)."""

    def __init__(self, nc, osb_pool, group=4):
        self.nc = nc
        self.osb_pool = osb_pool
        self.group = group
        self.cur = {}  # o_d name -> (tile, base_mt, count, o_d)

    def slot(self, o_d, mt):
        key = id(o_d)
        tile_, base, cnt, _ = self.cur.get(key, (None, None, 0, None))
        if tile_ is None or cnt == self.group or mt != base + cnt:
            self.flush(key)
            tile_ = self.osb_pool.tile([P, self.group, VD], F32, tag="ob", name="ob")
            self.cur[key] = (tile_, mt, 1, o_d)
            return tile_[:, 0, :]
        self.cur[key] = (tile_, base, cnt + 1, o_d)
        return tile_[:, cnt, :]

    def flush(self, key=None):
        if key is None:
            for k in list(self.cur):
                self.flush(k)
            return
        entry = self.cur.pop(key, None)
        if entry is None or entry[0] is None:
            return
        tile_, base, cnt, o_d = entry
        self.nc.sync.dma_start(
            out=o_d[128 * base : 128 * (base + cnt), :].rearrange(
                "(t p) d -> p t d", p=P
            ),
            in_=tile_[:, 0:cnt, :],
        )


def _emit_o_group(nc, po_pool, rc_pool, ob_batch, Esrc, ve, o_d, mt, nt):
    """One output tile: 16-deep PSUM accumulation + folded-softmax normalize."""
    pot = po_pool.tile([P, VD + 1], F32, tag="po", name="pot")
    for j in range(nt):
        nc.tensor.matmul(
            pot,
            lhsT=Esrc[:, j, 128 * mt : 128 * (mt + 1)],
            rhs=ve[:, j, 0 : VD + 1],
            start=(j == 0),
            stop=(j == nt - 1),
        )
    rc = rc_pool.tile([P, 1], F32, tag="rc", name="rct")
    nc.vector.reciprocal(rc, pot[:, VD : VD + 1])
    ob = ob_batch.slot(o_d, mt)
    nc.vector.tensor_scalar_mul(ob, pot[:, 0:VD], rc)


def _emit_body(nc, tc, consts, persist, dram, n, pools, warmup=True):
    """One full pass using caller-provided pools (single shared scope).

    Emission order is engine program order; anything placed before the score
    loop on PE/ACT/DVE must have its data ready early or it poisons the FIFO.
    k1-half2 prep and the v1 load are spliced INTO the score loop at steps
    where their DMAs have landed.
    """
    nt = n // P
    nch = n // 512
    half = nch // 2
    k1_d, k2_d, v1_d, v2_d, o1_d, o2_d = (
        dram["k1"], dram["k2"], dram["v1"], dram["v2"], dram["o1"], dram["o2"],
    )
    identity, W1_sb, b1_sb, W2_sb, b2_sb = consts

    k1pT = persist.tile([P, n], F32R, tag="k1pT", name="k1pT")
    k2pT = persist.tile([P, n], F32R, tag="k2pT", name="k2pT")
    E = persist.tile([P, nt, n], BF16, tag="E", name="E")
    ET = persist.tile([P, nt, n], BF16, tag="ET", name="ET")
    v1e = persist.tile([P, nt, VD + 2], BF16, tag="v1e", name="v1e")
    v2e = persist.tile([P, nt, VD + 2], BF16, tag="v2e", name="v2e")

    stage, ktbuf = pools["stage"], pools["ktbuf"]
    osb_pool, rc_pool = pools["osb"], pools["rc"]

    # PSUM pool lifetimes are staged (LIFO close order): po0 (2 banks) lives
    # for the whole body; ptpp (k-prep, 2 banks, innermost) closes once the
    # last k chunks are projected, freeing banks for po1; pscore (4 banks)
    # closes shortly after the score loop, freeing a deeper po2.  Output
    # groups rotate over the open po pools for deeper PSUM pipelining.
    po0_cm = tc.tile_pool(name="po0", bufs=2, space="PSUM")
    po0 = po0_cm.__enter__()
    pscore_cm = tc.tile_pool(name="pscore", bufs=2, space="PSUM")
    pscore = pscore_cm.__enter__()
    ptpp_cm = tc.tile_pool(name="ptpp", bufs=2, space="PSUM")
    ptpp = ptpp_cm.__enter__()
    po_pools = [po0]
    po1_cm = None
    n_groups = 0

    def emit_group(Esrc, ve, o_d, mt):
        nonlocal n_groups
        _emit_o_group(nc, po_pools[n_groups % len(po_pools)], rc_pool, ob_batch,
                      Esrc, ve, o_d, mt, nt)
        n_groups += 1

    def close_ptpp_open_po1():
        nonlocal po1_cm
        ptpp_cm.__exit__(None, None, None)
        po1_cm = tc.tile_pool(name="po1", bufs=2, space="PSUM")
        po_pools.append(po1_cm.__enter__())

    if warmup:
        # HAM warmup: dummy transposes keep the PE busy during the first
        # DMA wait so the ~3.4us cold-clock window burns throwaway work.
        pwarm = ptpp.tile([P, P], F32, tag="pt512", name="warm")
        for _ in range(12):
            nc.tensor.transpose(pwarm[:, 0:P], identity, identity)

    # k1 rows [0, n/2) + all of k2 -> score rows 0..nt/2-1 can start early;
    # v2 follows so v2e is ready for the first o2 groups.
    pre_half = half if half >= 1 else nch
    _emit_kprep(nc, stage, ktbuf, ptpp, k1_d, W1_sb, b1_sb, k1pT, identity,
                range(pre_half))
    _emit_kprep(nc, stage, ktbuf, ptpp, k2_d, W2_sb, b2_sb, k2pT, identity,
                range(nch))
    _emit_vload(nc, stage, v2_d, v2e, nch)

    w = min(1024, n)
    k1h2_at = max(2, nt // 4) if pre_half < nch else nt  # splice k1-half2 prep
    v1_at = nt // 2                                      # splice v1 load
    if k1h2_at >= nt:
        close_ptpp_open_po1()
    ob_batch = _OutBatcher(nc, osb_pool)
    n_o2_inline = 0
    for i in range(nt):
        if i == k1h2_at:
            _emit_kprep(nc, stage, ktbuf, ptpp, k1_d, W1_sb, b1_sb, k1pT,
                        identity, range(pre_half, nch))
            close_ptpp_open_po1()
        if i == v1_at:
            _emit_vload(nc, stage, v1_d, v1e, nch, eng="vector")
        for h in range(n // w):
            ps = pscore.tile([P, w], F32, tag="ps", name="ps")
            for q in range(w // 512):
                col = w * h + 512 * q
                nc.tensor.matmul(
                    ps[:, 512 * q : 512 * (q + 1)],
                    lhsT=k1pT[:, 128 * i : 128 * (i + 1)],
                    rhs=k2pT[:, col : col + 512],
                    start=True,
                    stop=True,
                )
            nc.scalar.activation(E[:, i, w * h : w * (h + 1)], ps, AF.Exp)
        # full row i of E is ready: xbar-transpose it into ET column-block i.
        # Issued from the SP ring: DMA completion-lane recycle waits then land
        # on SP.SEQ (latency-tolerant) instead of blocking ACT's exp stream.
        nc.sync.dma_start_transpose(
            out=ET[:, :, 128 * i : 128 * (i + 1)], in_=E[:, i, :]
        )
        if i >= O2_SLACK:
            emit_group(ET, v2e, o2_d, i - O2_SLACK)
            n_o2_inline += 1
    # o1 first: E is fully materialized at loop exit, so these never wait;
    # the tail slabs for the remaining o2 groups land during the o1 burst.
    # Two o1 groups go out on the old [po0, po1] rotation so the pscore/po1
    # pool-release barrier waits overlap real PE work, then the freed banks
    # reopen as po1b + a deeper po2.
    for mt in range(2):
        emit_group(E, v1e, o1_d, mt)
    po1_cm.__exit__(None, None, None)
    pscore_cm.__exit__(None, None, None)
    po1_cm = tc.tile_pool(name="po1b", bufs=2, space="PSUM")
    po2_cm = tc.tile_pool(name="po2", bufs=4, space="PSUM")
    po_pools = [po0, po2_cm.__enter__(), po1_cm.__enter__()]
    for mt in range(2, nt):
        emit_group(E, v1e, o1_d, mt)
    for mt in range(n_o2_inline, nt):
        emit_group(ET, v2e, o2_d, mt)
    ob_batch.flush()
    po1_cm.__exit__(None, None, None)
    po2_cm.__exit__(None, None, None)
    po0_cm.__exit__(None, None, None)


def _make_dram(nc, n):
    dram = {
        "k1": nc.dram_tensor("k1", [n, KD], F32, kind="ExternalInput").ap(),
        "k2": nc.dram_tensor("k2", [n, KD], F32, kind="ExternalInput").ap(),
        "v1": nc.dram_tensor("v1", [n, VD], F32, kind="ExternalInput").ap(),
        "v2": nc.dram_tensor("v2", [n, VD], F32, kind="ExternalInput").ap(),
        "o1": nc.dram_tensor("o1", [n, VD], F32, kind="ExternalOutput").ap(),
        "o2": nc.dram_tensor("o2", [n, VD], F32, kind="ExternalOutput").ap(),
    }
    W1_d = nc.dram_tensor("W1", [KD, AD], F32R, kind="ExternalInput").ap()
    b1_d = nc.dram_tensor("b1", [AD], F32, kind="ExternalInput").ap()
    W2_d = nc.dram_tensor("W2", [KD, AD], F32R, kind="ExternalInput").ap()
    b2_d = nc.dram_tensor("b2", [AD], F32, kind="ExternalInput").ap()
    return dram, (W1_d, b1_d, W2_d, b2_d)


def _load_consts(nc, consts_pool, wdram):
    W1_d, b1_d, W2_d, b2_d = wdram
    identity = consts_pool.tile([P, P], F32)
    make_identity(nc, identity)
    # consts go via SWDGE (gpsimd) so they don't serialize ahead of the
    # first k-tile loads on the sync sequencer
    W1_sb = consts_pool.tile([P, 2, AD], F32R)
    nc.gpsimd.dma_start(out=W1_sb, in_=W1_d.rearrange("(kb k) a -> k kb a", k=P))
    W2_sb = consts_pool.tile([P, 2, AD], F32R)
    nc.gpsimd.dma_start(out=W2_sb, in_=W2_d.rearrange("(kb k) a -> k kb a", k=P))
    b1_sb = consts_pool.tile([P, 1], F32)
    nc.gpsimd.dma_start(out=b1_sb, in_=b1_d.rearrange("(a one) -> a one", one=1))
    b2_sb = consts_pool.tile([P, 1], F32)
    nc.gpsimd.dma_start(out=b2_sb, in_=b2_d.rearrange("(a one) -> a one", one=1))
    return (identity, W1_sb, b1_sb, W2_sb, b2_sb)


def _make_pools(tc, ctx):
    return {
        "stage": ctx.enter_context(tc.tile_pool(name="stage", bufs=5)),
        "ktbuf": ctx.enter_context(tc.tile_pool(name="ktbuf", bufs=3)),
        "osb": ctx.enter_context(tc.tile_pool(name="osb", bufs=3)),
        "rc": ctx.enter_context(tc.tile_pool(name="rc", bufs=4)),
    }


def build_nc(n: int = N, reps: int = 1):
    """Single-shot SPMD program (what kernel() runs)."""
    import contextlib

    nc = bacc.Bacc("TRN2", target_bir_lowering=False, debug=False)
    dram, wdram = _make_dram(nc, n)
    with tile.TileContext(nc) as tc:
        with tc.tile_pool(name="consts", bufs=1) as consts_pool, tc.tile_pool(
            name="persist", bufs=1
        ) as persist, contextlib.ExitStack() as ctx:
            consts = _load_consts(nc, consts_pool, wdram)
            pools = _make_pools(tc, ctx)
            for _ in range(reps):
                _emit_body(nc, tc, consts, persist, dram, n, pools)
    nc.compile()
    return nc


def build_nc_loop(n: int = N, iters: int = 16):
    """Timing variant: whole body inside a hardware For_i loop."""
    import contextlib

    nc = bacc.Bacc("TRN2", target_bir_lowering=False, debug=False)
    dram, wdram = _make_dram(nc, n)
    with tile.TileContext(nc) as tc:
        with tc.tile_pool(name="consts", bufs=1) as consts_pool, tc.tile_pool(
            name="persist", bufs=1
        ) as persist, contextlib.ExitStack() as ctx:
            consts = _load_consts(nc, consts_pool, wdram)
            pools = _make_pools(tc, ctx)
            with tc.For_i(0, iters, 1):
                _emit_body(nc, tc, consts, persist, dram, n, pools,
                           warmup=False)
    nc.compile()
    return nc


_NC_CACHE: dict = {}


def _get_nc(n: int = N):
    if n not in _NC_CACHE:
        _NC_CACHE[n] = build_nc(n)
    return _NC_CACHE[n]


def kernel(k1, k2, v1, v2, W1, b1, W2, b2):
    """Full-input entry point: shard batch across 8 cores, run SPMD, gather."""
    nc = _get_nc(N)
    k1 = np.ascontiguousarray(np.asarray(k1, dtype=np.float32))
    k2 = np.ascontiguousarray(np.asarray(k2, dtype=np.float32))
    v1 = np.ascontiguousarray(np.asarray(v1, dtype=np.float32))
    v2 = np.ascontiguousarray(np.asarray(v2, dtype=np.float32))
    W1 = np.ascontiguousarray(np.asarray(W1, dtype=np.float32))
    b1 = np.ascontiguousarray(np.asarray(b1, dtype=np.float32))
    W2 = np.ascontiguousarray(np.asarray(W2, dtype=np.float32))
    b2 = np.ascontiguousarray(np.asarray(b2, dtype=np.float32))
    in_maps = [
        {
            "k1": k1[c], "k2": k2[c], "v1": v1[c], "v2": v2[c],
            "W1": W1, "b1": b1, "W2": W2, "b2": b2,
        }
        for c in range(N_CORES)
    ]
    res = bass_utils.run_bass_kernel_spmd(nc, in_maps, core_ids=list(range(N_CORES)))
    o2 = np.stack([res.results[c]["o2"] for c in range(N_CORES)])
    o1 = np.stack([res.results[c]["o1"] for c in range(N_CORES)])
    return (o2, o1)


# revision 64
# speedup vs baseline: 1.6409x; 1.6409x over previous
"""Bidirectional attention kernel for Trainium2 (8 NeuronCores, batch-parallel).

Math (per batch element, all on one core):
    k1p = k1 @ W1 + b1            [N, A]
    k2p = k2 @ W2 + b2            [N, A]
    S   = k1p @ k2p.T             [N, N]
    E   = exp(S)                  (no max-subtraction needed: |S| < ~25)
    o1[m, d] = sum_n E[n, m] v1[n, d] / sum_n E[n, m]   (softmax over N1)
    o2[n, d] = sum_m E[n, m] v2[m, d] / sum_m E[n, m]   (softmax over N2)

Schedule (v3 — xbar-transpose design; TimelineSim 98.0us, HW slope ~117ns/iter
vs baseline 145ns/iter):
  * ET (the transpose of E, needed for o2) is produced by 16 DMA xbar slab
    transposes (dma_start_transpose on the SP HWDGE ring), one per score
    row-tile, replacing 256 PE transposes + 80 DVE PSUM->SBUF copies
    (~14us of PE and ~32us of DVE work removed).  The SP ring is used so
    DMA completion-lane recycle waits land on SP.SEQ, not ACT's exp queue.
  * Input DMA order k1c0, k2 (c0..c3), k1c1, v2, k1[c2,c3], v1; score row
    0's first half runs while k2's tail chunks are still in flight, and
    every later prep is spliced into the score loop at a step where its
    DMA has landed (engine FIFOs never queue behind a distant DMA).
  * exp on ACT paces the score loop (~1147ns per 1024-wide strip); PE's
    spare time is filled by inline o2 output groups emitted as a budgeted
    js-stream (ROW_JS output matmuls per row keeps each row ACT-bound),
    chasing the slabs with O2_SLACK rows of slack and ending 2 rows early
    so their DVE chains drain before phase C.  Then all o1 groups (E is
    complete, zero waits), then the remaining o2 groups.
  * Projection bias-adds and output normalizes (reciprocal + scale of the
    folded ones-column denominator) run on DVE; ACT does exp only.
  * Output tiles are normalized into 4-tile batches and stored with one
    DMA per batch (HWDGE dispatch is ~625ns each); the final two o2 tiles
    store singly so the end-of-program barrier waits on a small DMA.
  * PSUM pools are lifetime-staged with LIFO close order: ptpp (k-prep)
    closes at the k1-half2 splice freeing banks for po1; pscore closes
    after the score loop freeing a 4-buf po2; output groups rotate over
    the open po pools.
"""

import numpy as np

import concourse.bass as bass
import concourse.tile as tile
from concourse import bacc, mybir, bass_utils
from concourse.masks import make_identity

N_CORES = 8
B = 8
N = 2048  # N1 == N2
KD = 256  # K1D == K2D
VD = 256  # V1D == V2D
AD = 128
P = 128

F32 = mybir.dt.float32
F32R = mybir.dt.float32r
BF16 = mybir.dt.bfloat16
AF = mybir.ActivationFunctionType

O2_SLACK = 6  # score rows
ROW_JS = 12  # inline o2 matmuls per score row


def _emit_kprep(nc, stage, ktbuf, ptpp, k_d, W_sb, b_sb, kpT, identity, chunks,
                copy_eng="vector"):
    """Load, PE-transpose, and project 512-row chunks of one k tensor.

    copy_eng="scalar" puts the PSUM->SBUF kt copies on ACT — useful for the
    prologue chunks while ACT is still idle (shortens the per-chunk
    arrival->transpose->copy->bias chain that gates the first score rows)."""
    ceng = getattr(nc, copy_eng)
    for c in chunks:
        st = stage.tile([P, 4, KD], F32, tag="stage", name="st")
        nc.sync.dma_start(
            out=st,
            in_=k_d[512 * c : 512 * (c + 1), :].rearrange("(t p) k -> p t k", p=P),
        )
        kt = ktbuf.tile([P, 2, 512], F32R, tag="kt", name="kt")
        for kb in range(2):
            pt = ptpp.tile([P, 512], F32, tag="pt512", name="pt")
            for t in range(4):
                nc.tensor.transpose(
                    pt[:, 128 * t : 128 * (t + 1)],
                    st[:, t, 128 * kb : 128 * (kb + 1)],
                    identity,
                )
            if copy_eng == "scalar":
                ceng.activation(kt[:, kb, :], pt, AF.Identity)
            else:
                ceng.tensor_copy(kt[:, kb, :], pt)
        pp = ptpp.tile([P, 512], F32, tag="pt512", name="pp")
        for kb in range(2):
            nc.tensor.matmul(
                pp, lhsT=W_sb[:, kb, :], rhs=kt[:, kb, :],
                start=(kb == 0), stop=(kb == 1),
            )
        nc.vector.tensor_scalar_add(kpT[:, 512 * c : 512 * (c + 1)], pp, b_sb)


def _emit_vload(nc, stage, v_d, ve, nch, eng="gpsimd"):
    """Load one v tensor into its bf16 extended tile (ones column at VD)."""
    nc.gpsimd.memset(ve[:, :, VD : VD + 2], 1.0)
    copy_eng = getattr(nc, eng)
    for c in range(nch):
        sv = stage.tile([P, 4, VD], F32, tag="stage", name="sv")
        nc.sync.dma_start(
            out=sv,
            in_=v_d[512 * c : 512 * (c + 1), :].rearrange("(t p) d -> p t d", p=P),
        )
        copy_eng.tensor_copy(ve[:, 4 * c : 4 * (c + 1), 0:VD], sv)


class _OutBatcher:
    """Accumulates normalized output tiles and stores them 4-at-a-time with a
    single DMA (HWDGE dispatch overhead is ~625ns; 8 big stores beat 32)."""

    def __init__(self, nc, osb_pool, group=4):
        self.nc = nc
        self.osb_pool = osb_pool
        self.group = group
        self.cur = {}  # o_d name -> (tile, base_mt, count, o_d)

    def slot(self, o_d, mt):
        key = id(o_d)
        tile_, base, cnt, _ = self.cur.get(key, (None, None, 0, None))
        if tile_ is None or cnt == self.group or mt != base + cnt:
            self.flush(key)
            tile_ = self.osb_pool.tile([P, self.group, VD], F32, tag="ob", name="ob")
            self.cur[key] = (tile_, mt, 1, o_d)
            return tile_[:, 0, :]
        self.cur[key] = (tile_, base, cnt + 1, o_d)
        return tile_[:, cnt, :]

    def flush(self, key=None):
        if key is None:
            for k in list(self.cur):
                self.flush(k)
            return
        entry = self.cur.pop(key, None)
        if entry is None or entry[0] is None:
            return
        tile_, base, cnt, o_d = entry
        self.nc.sync.dma_start(
            out=o_d[128 * base : 128 * (base + cnt), :].rearrange(
                "(t p) d -> p t d", p=P
            ),
            in_=tile_[:, 0:cnt, :],
        )


def _emit_o_mms(nc, pot, Esrc, ve, mt, j0, j1, nt):
    for j in range(j0, j1):
        nc.tensor.matmul(
            pot,
            lhsT=Esrc[:, j, 128 * mt : 128 * (mt + 1)],
            rhs=ve[:, j, 0 : VD + 1],
            start=(j == 0),
            stop=(j == nt - 1),
        )


def _emit_o_norm(nc, rc_pool, ob_batch, pot, o_d, mt):
    rc = rc_pool.tile([P, 1], F32, tag="rc", name="rct")
    nc.vector.reciprocal(rc, pot[:, VD : VD + 1])
    ob = ob_batch.slot(o_d, mt)
    nc.vector.tensor_scalar_mul(ob, pot[:, 0:VD], rc)


def _emit_o_group(nc, po_pool, rc_pool, ob_batch, Esrc, ve, o_d, mt, nt):
    """One output tile: 16-deep PSUM accumulation + folded-softmax normalize."""
    pot = po_pool.tile([P, VD + 1], F32, tag="po", name="pot")
    _emit_o_mms(nc, pot, Esrc, ve, mt, 0, nt, nt)
    _emit_o_norm(nc, rc_pool, ob_batch, pot, o_d, mt)


def _emit_body(nc, tc, consts, persist, dram, n, pools, warmup=True):
    """One full pass using caller-provided pools (single shared scope).

    Emission order is engine program order; anything placed before the score
    loop on PE/ACT/DVE must have its data ready early or it poisons the FIFO.
    k1-half2 prep and the v1 load are spliced INTO the score loop at steps
    where their DMAs have landed.
    """
    nt = n // P
    nch = n // 512
    half = nch // 2
    k1_d, k2_d, v1_d, v2_d, o1_d, o2_d = (
        dram["k1"], dram["k2"], dram["v1"], dram["v2"], dram["o1"], dram["o2"],
    )
    identity, W1_sb, b1_sb, W2_sb, b2_sb = consts

    k1pT = persist.tile([P, n], F32R, tag="k1pT", name="k1pT")
    k2pT = persist.tile([P, n], F32R, tag="k2pT", name="k2pT")
    E = persist.tile([P, nt, n], BF16, tag="E", name="E")
    ET = persist.tile([P, nt, n], BF16, tag="ET", name="ET")
    v1e = persist.tile([P, nt, VD + 2], BF16, tag="v1e", name="v1e")
    v2e = persist.tile([P, nt, VD + 2], BF16, tag="v2e", name="v2e")

    stage, ktbuf = pools["stage"], pools["ktbuf"]
    osb_pool, rc_pool = pools["osb"], pools["rc"]

    # PSUM pool lifetimes are staged (LIFO close order): po0 (2 banks) lives
    # for the whole body; ptpp (k-prep, 2 banks, innermost) closes once the
    # last k chunks are projected, freeing banks for po1; pscore (4 banks)
    # closes shortly after the score loop, freeing a deeper po2.  Output
    # groups rotate over the open po pools for deeper PSUM pipelining.
    po0_cm = tc.tile_pool(name="po0", bufs=2, space="PSUM")
    po0 = po0_cm.__enter__()
    pscore_cm = tc.tile_pool(name="pscore", bufs=2, space="PSUM")
    pscore = pscore_cm.__enter__()
    ptpp_cm = tc.tile_pool(name="ptpp", bufs=2, space="PSUM")
    ptpp = ptpp_cm.__enter__()
    po_pools = [po0]
    po1_cm = None
    n_groups = 0

    def emit_group(Esrc, ve, o_d, mt):
        nonlocal n_groups
        _emit_o_group(nc, po_pools[n_groups % len(po_pools)], rc_pool, ob_batch,
                      Esrc, ve, o_d, mt, nt)
        n_groups += 1

    def close_ptpp_open_po1():
        nonlocal po1_cm
        ptpp_cm.__exit__(None, None, None)
        po1_cm = tc.tile_pool(name="po1", bufs=2, space="PSUM")
        po_pools.append(po1_cm.__enter__())

    if warmup:
        # HAM warmup: dummy transposes keep the PE busy during the first
        # DMA wait so the ~3.4us cold-clock window burns throwaway work.
        pwarm = ptpp.tile([P, P], F32, tag="pt512", name="warm")
        for _ in range(12):
            nc.tensor.transpose(pwarm[:, 0:P], identity, identity)

    # Minimal prologue: k1 chunk 0 + k2 chunks 0,1 are all that score
    # (row 0, h=0) needs.  The rest of k2, the rest of k1, v2, and v1 are
    # spliced into the score loop at steps where their DMAs have landed, so
    # neither the PE nor the ACT FIFO ever queues behind a distant DMA.
    pre_half = half if half >= 1 else nch
    deep = nch == 4
    _emit_kprep(nc, stage, ktbuf, ptpp, k1_d, W1_sb, b1_sb, k1pT, identity,
                range(1 if deep else pre_half))
    _emit_kprep(nc, stage, ktbuf, ptpp, k2_d, W2_sb, b2_sb, k2pT, identity,
                range(2 if deep else nch))
    if not deep:
        _emit_vload(nc, stage, v2_d, v2e, nch)

    w = min(1024, n)
    k1h2_at = max(2, nt // 4) if pre_half < nch else nt  # splice k1-half2 prep
    v1_at = nt // 2                                      # splice v1 load
    if k1h2_at >= nt:
        close_ptpp_open_po1()
    ob_batch = _OutBatcher(nc, osb_pool)
    n_o2_inline = 0
    pending = None
    o2_queue = []

    def emit_score_strip(i, h):
        ps = pscore.tile([P, w], F32, tag="ps", name="ps")
        for q in range(w // 512):
            col = w * h + 512 * q
            nc.tensor.matmul(
                ps[:, 512 * q : 512 * (q + 1)],
                lhsT=k1pT[:, 128 * i : 128 * (i + 1)],
                rhs=k2pT[:, col : col + 512],
                start=True,
                stop=True,
            )
        nc.scalar.activation(E[:, i, w * h : w * (h + 1)], ps, AF.Exp)

    start_row = 0
    if deep:
        # rest of k2 between row 0's halves; k1 chunk 1 and v2 after row 0
        start_row = 1
        emit_score_strip(0, 0)
        _emit_kprep(nc, stage, ktbuf, ptpp, k2_d, W2_sb, b2_sb, k2pT, identity,
                    range(2, 4))
        emit_score_strip(0, 1)
        nc.sync.dma_start_transpose(out=ET[:, :, 0:128], in_=E[:, 0, :])
        _emit_kprep(nc, stage, ktbuf, ptpp, k1_d, W1_sb, b1_sb, k1pT, identity,
                    range(1, pre_half))
        _emit_vload(nc, stage, v2_d, v2e, nch)
    for i in range(start_row, nt):
        if i == k1h2_at:
            _emit_kprep(nc, stage, ktbuf, ptpp, k1_d, W1_sb, b1_sb, k1pT,
                        identity, range(pre_half, nch))
            close_ptpp_open_po1()
        if i == v1_at:
            _emit_vload(nc, stage, v1_d, v1e, nch, eng="vector")
        for h in range(n // w):
            emit_score_strip(i, h)
        # full row i of E is ready: xbar-transpose it into ET column-block i.
        # Issued from the SP ring: DMA completion-lane recycle waits then land
        # on SP.SEQ (latency-tolerant) instead of blocking ACT's exp stream.
        nc.sync.dma_start_transpose(
            out=ET[:, :, 128 * i : 128 * (i + 1)], in_=E[:, i, :]
        )
        # inline o2 groups as a budgeted js-stream: at most ROW_JS output
        # matmuls per score row keeps each row ACT-bound (row budget =
        # exp pace 2294ns - score mms 852ns ~= 13 x 107ns)
        # stop queueing 2 rows early so the last inline groups' DVE chains
        # (recip+tsmul -> po/osb frees) drain before phase C's first group
        if O2_SLACK <= i < nt - 2:
            o2_queue.append(i - O2_SLACK)
        budget = ROW_JS
        while budget > 0 and (pending is not None or o2_queue):
            if pending is None:
                mt_c = o2_queue.pop(0)
                pot_c = po_pools[n_groups % len(po_pools)].tile(
                    [P, VD + 1], F32, tag="po", name="pot"
                )
                n_groups += 1
                pending = [pot_c, mt_c, 0]
            pot_p, mt_p, j0 = pending
            j1 = min(nt, j0 + budget)
            _emit_o_mms(nc, pot_p, ET, v2e, mt_p, j0, j1, nt)
            budget -= j1 - j0
            if j1 == nt:
                _emit_o_norm(nc, rc_pool, ob_batch, pot_p, o2_d, mt_p)
                pending = None
                n_o2_inline += 1
            else:
                pending[2] = j1
    if pending is not None:
        pot_p, mt_p, j0 = pending
        _emit_o_mms(nc, pot_p, ET, v2e, mt_p, j0, nt, nt)
        _emit_o_norm(nc, rc_pool, ob_batch, pot_p, o2_d, mt_p)
        pending = None
        n_o2_inline += 1
    # o1 first: E is fully materialized at loop exit, so these never wait;
    # the tail slabs for the remaining o2 groups land during the o1 burst.
    # Two o1 groups go out on the old [po0, po1] rotation so the pscore/po1
    # pool-release barrier waits overlap real PE work, then the freed banks
    # reopen as po1b + a deeper po2.
    for mt in range(2):
        emit_group(E, v1e, o1_d, mt)
    po1_cm.__exit__(None, None, None)
    pscore_cm.__exit__(None, None, None)
    po1_cm = tc.tile_pool(name="po1b", bufs=2, space="PSUM")
    po2_cm = tc.tile_pool(name="po2", bufs=4, space="PSUM")
    po_pools = [po0, po2_cm.__enter__(), po1_cm.__enter__()]
    for mt in range(2, nt):
        emit_group(E, v1e, o1_d, mt)
    # last two o2 tiles store as singles so the final DMA is small and the
    # end-of-program barrier isn't gated on a 4-tile store
    tail_batch = _OutBatcher(nc, osb_pool, group=1)
    for mt in range(n_o2_inline, nt - 2):
        emit_group(ET, v2e, o2_d, mt)
    ob_batch.flush()
    for mt in range(max(n_o2_inline, nt - 2), nt):
        _emit_o_group(nc, po_pools[n_groups % len(po_pools)], rc_pool,
                      tail_batch, ET, v2e, o2_d, mt, nt)
        n_groups += 1
    tail_batch.flush()
    po1_cm.__exit__(None, None, None)
    po2_cm.__exit__(None, None, None)
    po0_cm.__exit__(None, None, None)


def _make_dram(nc, n):
    dram = {
        "k1": nc.dram_tensor("k1", [n, KD], F32, kind="ExternalInput").ap(),
        "k2": nc.dram_tensor("k2", [n, KD], F32, kind="ExternalInput").ap(),
        "v1": nc.dram_tensor("v1", [n, VD], F32, kind="ExternalInput").ap(),
        "v2": nc.dram_tensor("v2", [n, VD], F32, kind="ExternalInput").ap(),
        "o1": nc.dram_tensor("o1", [n, VD], F32, kind="ExternalOutput").ap(),
        "o2": nc.dram_tensor("o2", [n, VD], F32, kind="ExternalOutput").ap(),
    }
    W1_d = nc.dram_tensor("W1", [KD, AD], F32R, kind="ExternalInput").ap()
    b1_d = nc.dram_tensor("b1", [AD], F32, kind="ExternalInput").ap()
    W2_d = nc.dram_tensor("W2", [KD, AD], F32R, kind="ExternalInput").ap()
    b2_d = nc.dram_tensor("b2", [AD], F32, kind="ExternalInput").ap()
    return dram, (W1_d, b1_d, W2_d, b2_d)


def _load_consts(nc, consts_pool, wdram):
    W1_d, b1_d, W2_d, b2_d = wdram
    identity = consts_pool.tile([P, P], F32)
    make_identity(nc, identity)
    # consts go via SWDGE (gpsimd) so they don't serialize ahead of the
    # first k-tile loads on the sync sequencer
    W1_sb = consts_pool.tile([P, 2, AD], F32R)
    nc.gpsimd.dma_start(out=W1_sb, in_=W1_d.rearrange("(kb k) a -> k kb a", k=P))
    W2_sb = consts_pool.tile([P, 2, AD], F32R)
    nc.gpsimd.dma_start(out=W2_sb, in_=W2_d.rearrange("(kb k) a -> k kb a", k=P))
    b1_sb = consts_pool.tile([P, 1], F32)
    nc.gpsimd.dma_start(out=b1_sb, in_=b1_d.rearrange("(a one) -> a one", one=1))
    b2_sb = consts_pool.tile([P, 1], F32)
    nc.gpsimd.dma_start(out=b2_sb, in_=b2_d.rearrange("(a one) -> a one", one=1))
    return (identity, W1_sb, b1_sb, W2_sb, b2_sb)


def _make_pools(tc, ctx):
    return {
        "stage": ctx.enter_context(tc.tile_pool(name="stage", bufs=5)),
        "ktbuf": ctx.enter_context(tc.tile_pool(name="ktbuf", bufs=3)),
        "osb": ctx.enter_context(tc.tile_pool(name="osb", bufs=3)),
        "rc": ctx.enter_context(tc.tile_pool(name="rc", bufs=4)),
    }


def build_nc(n: int = N, reps: int = 1):
    """Single-shot SPMD program (what kernel() runs)."""
    import contextlib

    nc = bacc.Bacc("TRN2", target_bir_lowering=False, debug=False)
    dram, wdram = _make_dram(nc, n)
    with tile.TileContext(nc) as tc:
        with tc.tile_pool(name="consts", bufs=1) as consts_pool, tc.tile_pool(
            name="persist", bufs=1
        ) as persist, contextlib.ExitStack() as ctx:
            consts = _load_consts(nc, consts_pool, wdram)
            pools = _make_pools(tc, ctx)
            for _ in range(reps):
                _emit_body(nc, tc, consts, persist, dram, n, pools)
    nc.compile()
    return nc


def build_nc_loop(n: int = N, iters: int = 16):
    """Timing variant: whole body inside a hardware For_i loop."""
    import contextlib

    nc = bacc.Bacc("TRN2", target_bir_lowering=False, debug=False)
    dram, wdram = _make_dram(nc, n)
    with tile.TileContext(nc) as tc:
        with tc.tile_pool(name="consts", bufs=1) as consts_pool, tc.tile_pool(
            name="persist", bufs=1
        ) as persist, contextlib.ExitStack() as ctx:
            consts = _load_consts(nc, consts_pool, wdram)
            pools = _make_pools(tc, ctx)
            with tc.For_i(0, iters, 1):
                _emit_body(nc, tc, consts, persist, dram, n, pools,
                           warmup=False)
    nc.compile()
    return nc


_NC_CACHE: dict = {}


def _get_nc(n: int = N):
    if n not in _NC_CACHE:
        _NC_CACHE[n] = build_nc(n)
    return _NC_CACHE[n]


def kernel(k1, k2, v1, v2, W1, b1, W2, b2):
    """Full-input entry point: shard batch across 8 cores, run SPMD, gather."""
    nc = _get_nc(N)
    k1 = np.ascontiguousarray(np.asarray(k1, dtype=np.float32))
    k2 = np.ascontiguousarray(np.asarray(k2, dtype=np.float32))
    v1 = np.ascontiguousarray(np.asarray(v1, dtype=np.float32))
    v2 = np.ascontiguousarray(np.asarray(v2, dtype=np.float32))
    W1 = np.ascontiguousarray(np.asarray(W1, dtype=np.float32))
    b1 = np.ascontiguousarray(np.asarray(b1, dtype=np.float32))
    W2 = np.ascontiguousarray(np.asarray(W2, dtype=np.float32))
    b2 = np.ascontiguousarray(np.asarray(b2, dtype=np.float32))
    in_maps = [
        {
            "k1": k1[c], "k2": k2[c], "v1": v1[c], "v2": v2[c],
            "W1": W1, "b1": b1, "W2": W2, "b2": b2,
        }
        for c in range(N_CORES)
    ]
    res = bass_utils.run_bass_kernel_spmd(nc, in_maps, core_ids=list(range(N_CORES)))
    o2 = np.stack([res.results[c]["o2"] for c in range(N_CORES)])
    o1 = np.stack([res.results[c]["o1"] for c in range(N_CORES)])
    return (o2, o1)

